# revision 18
# baseline (speedup 1.0000x reference)
"""Trainium2 Bass kernel for nn_BertNerHF (BERT encoder + NER head with
valid-token stream compaction).

Distribution: data-parallel over the batch (B=4 rows). Each pair of cores
(2b, 2b+1) holds row b; both compute the full row (duplicated pair), outputs
taken from the even cores. No cross-core communication.

On-core layout: activations are kept FEATURE-major in SBUF (xT: [D partitions
(6 tiles of 128), S tokens free]) so that
  - every GEMM is matmul(psum, lhsT=W[kc,kf], rhs=xT[kc, :]) with weights in
    their natural [in,out] layout,
  - per-feature biases / LN gains are per-partition scalars (tensor_scalar),
  - LN token-statistics are computed with ones-column matmuls on the PE and
    re-broadcast across partitions with gpsimd partition_broadcast.
Attention: scoresT (k-major) from lhsT=KT-head, rhs=QT-head; exp fused with
the PSUM eviction on ScalarE (scale=1/8, per-key mask bias); ctx accumulated
as lhsT=V_aug (token-major V with an appended ones column, so row 64 of the
PSUM result is the softmax denominator), then normalized during eviction.
Final compaction is a permutation matmul with a host-built 0/1 matrix (row
512 routes the softmax(cls_b) padding row).

PSUM budget (8 banks, statically reserved per pool tag):
  a(2) b(2) c(2): rotating GEMM/score/ctx/transpose tiles; W2 runs two
  3-bank passes across a/b/c; d(1)/e(1): LN stat rows s1/s2.
"""

import math
from contextlib import ExitStack

import ml_dtypes
import numpy as np

import bass_rust
import concourse.bass as bass
import concourse.mybir as mybir
import concourse.tile as tile
from concourse.bass_utils import run_bass_kernel_spmd

B, S, D, L, H, V, NL = 4, 512, 768, 4, 12, 30522, 9
DH = D // H          # 64
DF = 4 * D           # 3072
P = 128
KD = D // P          # 6  k-tiles over D
KF = DF // P         # 24 k-tiles over DF
NT = S // P          # 4  token tiles
FP = mybir.dt.float32
BF = mybir.dt.bfloat16
BF_NP = ml_dtypes.bfloat16
AF = mybir.ActivationFunctionType

_MAX_WAITS_PER_INST = 1


def _patched_drain_and_barrier(self, tick_clock, wait_clock):
    """The nix walrus build rejects multi-wait TPB_CTRL (Drain) instructions
    ("Too many sync wait commands"); split the tail drain's waits across
    multiple Drain instructions."""
    from concourse.tile import ScopedClock

    nc = self.nc
    drain_inst = nc.sync.drain()
    wait_clock.add_sem_waits(
        drain_inst.ins, ScopedClock({None: tick_clock.global_clock})
    )
    si = drain_inst.ins.sync_info
    waits = list(si.on_wait or [])
    if len(waits) > _MAX_WAITS_PER_INST:
        drain_inst.ins.sync_info = bass_rust.SyncInfo(
            on_wait=waits[:_MAX_WAITS_PER_INST],
            on_update=list(si.on_update or []),
        )
        for i in range(_MAX_WAITS_PER_INST, len(waits), _MAX_WAITS_PER_INST):
            extra = nc.sync.drain()
            extra.ins.sync_info = bass_rust.SyncInfo(
                on_wait=waits[i : i + _MAX_WAITS_PER_INST], on_update=[]
            )

    nc.all_engine_barrier()
    popped = nc._tile_sem_poison_stack.pop()
    assert popped is self._sem_poison
    nc.clear_and_free_semaphores(list(self.sems.allocated().values()))
    nc.all_engine_barrier()


tile.TileContext._drain_and_barrier = _patched_drain_and_barrier

_MAX_WAITS_GENERIC = 1


def _split_waits(nc, max_waits=_MAX_WAITS_GENERIC):
    """Split multi-wait engine instructions: the nix walrus codegen rejects
    instructions carrying more than a couple of semaphore waits. Excess waits
    move to no-op carrier instructions inserted just before, on the same
    engine. DMA/queue instructions are skipped (their waits are observed by
    the DGE, not the engine sequencer)."""
    # snapshot every block's list BEFORE creating carrier nops (nop() appends
    # to the current block; final reassignment drops those stray copies)
    snaps = [(bb, list(bb.instructions)) for bb in nc.main_func.blocks]

    def needs_split(inst):
        si = inst.sync_info
        return si is not None and len(si.on_wait or []) > max_waits

    new_lists = []
    for bb, insts in snaps:
        new_list = []
        for inst in insts:
            if needs_split(inst):
                si = inst.sync_info
                waits = list(si.on_wait or [])
                excess = waits[:-max_waits]
                eng = nc.engines[inst.engine]
                for j in range(0, len(excess), max_waits):
                    carrier = eng.drain().ins
                    carrier.sync_info = bass_rust.SyncInfo(
                        on_wait=excess[j:j + max_waits], on_update=[])
                    new_list.append(carrier)
                inst.sync_info = bass_rust.SyncInfo(
                    on_wait=waits[-max_waits:],
                    on_update=list(si.on_update or []))
            new_list.append(inst)
        new_lists.append((bb, new_list))
    for bb, new_list in new_lists:
        bb.instructions = new_list


def build_nc(debug_taps=False):
    nc = bass.Bass(trn_type="TRN2", debug=False, num_devices=8)

    # ---- I/O -------------------------------------------------------------
    ios = dict(
        ids=nc.dram_tensor("ids", [S, 1], mybir.dt.int32, kind="ExternalInput"),
        wemb=nc.dram_tensor("wemb", [V, D], BF, kind="ExternalInput"),
        posT=nc.dram_tensor("posT", [D, S], BF, kind="ExternalInput"),
        kbias=nc.dram_tensor("kbias", [S, 1], FP, kind="ExternalInput"),
        emb_g=nc.dram_tensor("emb_g", [D, 1], FP, kind="ExternalInput"),
        emb_b=nc.dram_tensor("emb_b", [D, 1], FP, kind="ExternalInput"),
        wq=nc.dram_tensor("wq", [L, D, D], BF, kind="ExternalInput"),
        wk=nc.dram_tensor("wk", [L, D, D], BF, kind="ExternalInput"),
        wv=nc.dram_tensor("wv", [L, D, D], BF, kind="ExternalInput"),
        wo=nc.dram_tensor("wo", [L, D, D], BF, kind="ExternalInput"),
        w1=nc.dram_tensor("w1", [L, D, DF], BF, kind="ExternalInput"),
        w2=nc.dram_tensor("w2", [L, DF, D], BF, kind="ExternalInput"),
        bq=nc.dram_tensor("bq", [L, D, 1], FP, kind="ExternalInput"),
        bk=nc.dram_tensor("bk", [L, D, 1], FP, kind="ExternalInput"),
        bv=nc.dram_tensor("bv", [L, 1, D], FP, kind="ExternalInput"),
        bo=nc.dram_tensor("bo", [L, D, 1], FP, kind="ExternalInput"),
        b1=nc.dram_tensor("b1", [L, DF, 1], FP, kind="ExternalInput"),
        b2=nc.dram_tensor("b2", [L, D, 1], FP, kind="ExternalInput"),
        ln1g=nc.dram_tensor("ln1g", [L, D, 1], FP, kind="ExternalInput"),
        ln1b=nc.dram_tensor("ln1b", [L, D, 1], FP, kind="ExternalInput"),
        ln2g=nc.dram_tensor("ln2g", [L, D, 1], FP, kind="ExternalInput"),
        ln2b=nc.dram_tensor("ln2b", [L, D, 1], FP, kind="ExternalInput"),
        clsw=nc.dram_tensor("clsw", [D, NL], BF, kind="ExternalInput"),
        clsb=nc.dram_tensor("clsb", [NL, 1], FP, kind="ExternalInput"),
        clsb_row=nc.dram_tensor("clsb_row", [1, NL], FP, kind="ExternalInput"),
        pmT=nc.dram_tensor("pmT", [5 * P, S], BF, kind="ExternalInput"),
        outp=nc.dram_tensor("out", [S, NL], FP, kind="ExternalOutput"),
    )
    taps = {}
    if debug_taps:
        for nm in ["x0", "x1", "x2", "x3", "x4"]:
            taps[nm] = nc.dram_tensor("tap_" + nm, [P, KD, S], FP,
                                      kind="ExternalOutput")
        taps["logitsT"] = nc.dram_tensor("tap_logitsT", [NL, S], FP,
                                         kind="ExternalOutput")

    with tile.TileContext(nc) as tc:
        _build_body(nc, tc, ios, debug_taps, taps)
    _split_waits(nc)
    return nc


def _build_body(nc, tc, t, debug_taps, taps):
    with ExitStack() as ctx:
        const = ctx.enter_context(tc.tile_pool(name="const", bufs=1))
        act = ctx.enter_context(tc.tile_pool(name="act", bufs=1))
        wt = ctx.enter_context(tc.tile_pool(name="wt", bufs=1))
        misc = ctx.enter_context(tc.tile_pool(name="misc", bufs=1))
        ps = ctx.enter_context(tc.tile_pool(name="ps", bufs=1, space="PSUM"))

        # ---- constants ---------------------------------------------------
        ident_bf = const.tile([P, P], BF)
        from concourse.masks import make_identity
        make_identity(nc, ident_bf[:])
        ident_fp = const.tile([P, P], FP)
        make_identity(nc, ident_fp[:])
        ones_col = const.tile([P, 1], BF)
        nc.vector.memset(ones_col[:], 1.0)
        ones_row = const.tile([1, P], FP)
        nc.vector.memset(ones_row[:], 1.0)
        eps_t = const.tile([1, 1], FP)
        nc.vector.memset(eps_t[:], 1e-12)

        posT_sb = wt.tile([P, KD, S], BF, name="bigscratch")
        nc.sync.dma_start(out=posT_sb[:],
                          in_=t["posT"].rearrange("(c p) s -> p c s", p=P))
        kb_sb = const.tile([P, NT], FP)
        nc.sync.dma_start(out=kb_sb[:],
                          in_=t["kbias"].rearrange("(t p) 1 -> p t", p=P))
        ids_sb = const.tile([P, NT], mybir.dt.int32)
        nc.sync.dma_start(out=ids_sb[:],
                          in_=t["ids"].rearrange("(t p) 1 -> p t", p=P))
        embg_sb = const.tile([P, KD], FP)
        nc.sync.dma_start(out=embg_sb[:],
                          in_=t["emb_g"].rearrange("(c p) 1 -> p c", p=P))
        embb_sb = const.tile([P, KD], FP)
        nc.sync.dma_start(out=embb_sb[:],
                          in_=t["emb_b"].rearrange("(c p) 1 -> p c", p=P))

        # ---- embedding: gather + transpose + pos/type + LN ---------------
        we_t = []
        for ti in range(NT):
            g = misc.tile([P, D], BF, name="wegather", bufs=4)
            nc.gpsimd.indirect_dma_start(
                out=g[:], out_offset=None, in_=t["wemb"][:, :],
                in_offset=bass.IndirectOffsetOnAxis(ap=ids_sb[:, ti:ti + 1],
                                                    axis=0))
            we_t.append(g)
        z = act.tile([P, KD, S], BF, name="z", bufs=2)
        for f in range(KD):
            pst = ps.tile([P, S], BF, name="a", bufs=2)
            for ti in range(NT):
                nc.tensor.transpose(out=pst[:, ti * P:(ti + 1) * P],
                                    in_=we_t[ti][:, f * P:(f + 1) * P],
                                    identity=ident_bf[:])
            nc.vector.tensor_add(out=z[:, f, :], in0=pst[:],
                                 in1=posT_sb[:, f, :])

        def layer_norm(zt, g_sb, b_sb, name):
            """LN over the feature (partition) dim of zt [P, KD, S] -> bf16."""
            s1 = ps.tile([1, S], FP, name="d", bufs=1)
            s2 = ps.tile([1, S], FP, name="e", bufs=1)
            for c in range(KD):
                nc.tensor.matmul(out=s1[:], lhsT=ones_col[:], rhs=zt[:, c, :],
                                 start=(c == 0), stop=(c == KD - 1))
                sq = misc.tile([P, S], BF, name="sqs", bufs=2)
                nc.vector.tensor_mul(out=sq[:], in0=zt[:, c, :],
                                     in1=zt[:, c, :])
                nc.tensor.matmul(out=s2[:], lhsT=ones_col[:], rhs=sq[:],
                                 start=(c == 0), stop=(c == KD - 1))
            m2 = misc.tile([1, S], FP, name="m2")
            nc.scalar.activation(out=m2[:], in_=s1[:], func=AF.Square,
                                 scale=1.0 / math.sqrt(D))
            u = misc.tile([1, S], FP, name="u")
            nc.vector.tensor_tensor(out=u[:], in0=s2[:], in1=m2[:],
                                    op=mybir.AluOpType.subtract)
            r = misc.tile([1, S], FP, name="r")
            nc.scalar.activation(out=r[:], in_=u[:], func=AF.Sqrt,
                                 bias=eps_t[:], scale=1.0 / D)
            rstd = misc.tile([1, S], FP, name="rstd")
            nc.vector.reciprocal(out=rstd[:], in_=r[:])
            mu = misc.tile([1, S], FP, name="mu")
            nc.scalar.mul(out=mu[:], in_=s1[:], mul=1.0 / D)
            mu_b = ps.tile([P, S], FP, name="d", bufs=1)
            nc.tensor.matmul(out=mu_b[:], lhsT=ones_row[:], rhs=mu[:],
                             start=True, stop=True)
            rstd_b = ps.tile([P, S], FP, name="e", bufs=1)
            nc.tensor.matmul(out=rstd_b[:], lhsT=ones_row[:], rhs=rstd[:],
                             start=True, stop=True)
            xo = act.tile([P, KD, S], BF, name=name, bufs=2)
            for c in range(KD):
                tt = misc.tile([P, S], FP, name="lnt", bufs=2)
                nc.vector.tensor_tensor(out=tt[:], in0=zt[:, c, :],
                                        in1=mu_b[:],
                                        op=mybir.AluOpType.subtract)
                nc.vector.tensor_mul(out=tt[:], in0=tt[:], in1=rstd_b[:])
                nc.vector.tensor_scalar(
                    out=xo[:, c, :], in0=tt[:],
                    scalar1=g_sb[:, c:c + 1], scalar2=b_sb[:, c:c + 1],
                    op0=mybir.AluOpType.mult, op1=mybir.AluOpType.add)
            return xo

        xT = layer_norm(z, embg_sb, embb_sb, "xT")
        if debug_taps:
            dbg = act.tile([P, KD, S], FP, name="dbg", bufs=1)
            for c in range(KD):
                nc.vector.tensor_copy(out=dbg[:, c, :], in_=xT[:, c, :])
            nc.sync.dma_start(out=taps["x0"][:], in_=dbg[:])

        def col_bias(src, ncols, name):
            b_sb = misc.tile([P, ncols], FP, name=name)
            nc.sync.dma_start(out=b_sb[:],
                              in_=src.rearrange("(c p) 1 -> p c", p=P))
            return b_sb

        # ---- transformer layers ------------------------------------------
        for l in range(L):
            bq_sb = col_bias(t["bq"][l], KD, "bq_sb")
            bk_sb = col_bias(t["bk"][l], KD, "bk_sb")
            bo_sb = col_bias(t["bo"][l], KD, "bo_sb")
            b2_sb = col_bias(t["b2"][l], KD, "b2_sb")
            b1_sb = col_bias(t["b1"][l], KF, "b1_sb")
            l1g_sb = col_bias(t["ln1g"][l], KD, "l1g_sb")
            l1b_sb = col_bias(t["ln1b"][l], KD, "l1b_sb")
            l2g_sb = col_bias(t["ln2g"][l], KD, "l2g_sb")
            l2b_sb = col_bias(t["ln2b"][l], KD, "l2b_sb")
            bv_row = misc.tile([1, D], FP, name="bv_row")
            nc.sync.dma_start(out=bv_row[:], in_=t["bv"][l])

            # Q/K projections -> feature-major QT/KT [P, KD, S]
            wq_sb = wt.tile([P, KD, D], BF, name="wx", bufs=2)
            nc.sync.dma_start(out=wq_sb[:],
                              in_=t["wq"][l].rearrange("(c p) f -> p c f", p=P))
            wk_sb = wt.tile([P, KD, D], BF, name="wx", bufs=2)
            nc.sync.dma_start(out=wk_sb[:],
                              in_=t["wk"][l].rearrange("(c p) f -> p c f", p=P))
            qT = act.tile([P, KD, S], BF, name="qT")
            kT = act.tile([P, KD, S], BF, name="kT")
            for dst, w_sb, b_sb in ((qT, wq_sb, bq_sb), (kT, wk_sb, bk_sb)):
                for f in range(KD):
                    pst = ps.tile([P, S], FP, name="a", bufs=2)
                    for c in range(KD):
                        nc.tensor.matmul(out=pst[:],
                                         lhsT=w_sb[:, c, f * P:(f + 1) * P],
                                         rhs=xT[:, c, :],
                                         start=(c == 0), stop=(c == KD - 1))
                    nc.vector.tensor_scalar_add(out=dst[:, f, :], in0=pst[:],
                                                scalar1=b_sb[:, f:f + 1])

            # V projection -> token-major V_aug [P(tok), NT, H, DH+1]
            wv_sb = wt.tile([P, KD, D], BF, name="wx", bufs=2)
            nc.sync.dma_start(out=wv_sb[:],
                              in_=t["wv"][l].rearrange("(c p) f -> p c f", p=P))
            va = act.tile([P, NT, H, DH + 1], BF, name="va")
            nc.vector.memset(va[:, :, :, DH:DH + 1], 1.0)
            for ti in range(NT):
                for fb in range(2):
                    pst = ps.tile([P, 384], FP, name="b", bufs=2)
                    nc.tensor.matmul(
                        out=pst[:],
                        lhsT=ones_row[:],
                        rhs=bv_row[:, fb * 384:(fb + 1) * 384],
                        start=True, stop=False)
                    for c in range(KD):
                        nc.tensor.matmul(
                            out=pst[:],
                            lhsT=xT[:, c, ti * P:(ti + 1) * P],
                            rhs=wv_sb[:, c, fb * 384:(fb + 1) * 384],
                            start=False, stop=(c == KD - 1))
                    nc.vector.tensor_copy(
                        out=va[:, ti, fb * 6:(fb + 1) * 6, 0:DH],
                        in_=pst.rearrange("p (a b) -> p a b", a=6))

            # attention per head
            ctxT = act.tile([P, KD, S], BF, name="ctxT")
            for h in range(H):
                hp = (h % 2) * DH
                hf = h // 2
                expT = act.tile([P, NT, S], BF, name="expT", bufs=2)
                for kt in range(NT):
                    ps_s = ps.tile([P, S], FP, name="b", bufs=2)
                    nc.tensor.matmul(
                        out=ps_s[:],
                        lhsT=kT[hp:hp + DH, hf, kt * P:(kt + 1) * P],
                        rhs=qT[hp:hp + DH, hf, :],
                        start=True, stop=True)
                    nc.scalar.activation(
                        out=expT[:, kt, :], in_=ps_s[:], func=AF.Exp,
                        scale=1.0 / math.sqrt(DH),
                        bias=kb_sb[:, kt:kt + 1])
                ps_c = ps.tile([P, S], FP, name="c", bufs=2)
                for kt in range(NT):
                    nc.tensor.matmul(out=ps_c[:DH + 1, :],
                                     lhsT=va[:, kt, h, :],
                                     rhs=expT[:, kt, :],
                                     start=(kt == 0), stop=(kt == NT - 1))
                rec = misc.tile([1, S], FP, name="rec", bufs=2)
                nc.vector.reciprocal(out=rec[:], in_=ps_c[DH:DH + 1, :])
                rec_b = ps.tile([P, S], FP, name="b", bufs=2)
                nc.tensor.matmul(out=rec_b[:DH, :],
                                 lhsT=ones_row[:, :DH], rhs=rec[:],
                                 start=True, stop=True)
                craw = misc.tile([DH, S], FP, name="craw", bufs=2)
                nc.vector.tensor_copy(out=craw[:], in_=ps_c[:DH, :])
                nc.vector.tensor_tensor(out=ctxT[hp:hp + DH, hf, :],
                                        in0=craw[:], in1=rec_b[:DH, :],
                                        op=mybir.AluOpType.mult)

            # attention output projection + residual + LN1
            wo_sb = wt.tile([P, KD, D], BF, name="wx", bufs=2)
            nc.sync.dma_start(out=wo_sb[:],
                              in_=t["wo"][l].rearrange("(c p) f -> p c f", p=P))
            z1 = act.tile([P, KD, S], BF, name="z", bufs=2)
            for f in range(KD):
                pst = ps.tile([P, S], FP, name="a", bufs=2)
                for c in range(KD):
                    nc.tensor.matmul(out=pst[:],
                                     lhsT=wo_sb[:, c, f * P:(f + 1) * P],
                                     rhs=ctxT[:, c, :],
                                     start=(c == 0), stop=(c == KD - 1))
                nc.vector.tensor_scalar_add(out=pst[:], in0=pst[:],
                                            scalar1=bo_sb[:, f:f + 1])
                nc.vector.tensor_tensor(out=z1[:, f, :], in0=pst[:],
                                        in1=xT[:, f, :],
                                        op=mybir.AluOpType.add)
            x1 = layer_norm(z1, l1g_sb, l1b_sb, "xT")

            # FFN up-projection + gelu, W1 streamed in two halves
            hT = act.tile([P, KF, S], BF, name="hT")
            for half in range(2):
                w1_sb = wt.tile([P, KD, DF // 2], BF, name="w1h", bufs=1)
                nc.sync.dma_start(
                    out=w1_sb[:],
                    in_=t["w1"][l].rearrange("(c p) f -> p c f", p=P)[
                        :, :, half * (DF // 2):(half + 1) * (DF // 2)])
                for fi in range(KF // 2):
                    f = half * (KF // 2) + fi
                    pst = ps.tile([P, S], FP, name="a", bufs=2)
                    for c in range(KD):
                        nc.tensor.matmul(out=pst[:],
                                         lhsT=w1_sb[:, c, fi * P:(fi + 1) * P],
                                         rhs=x1[:, c, :],
                                         start=(c == 0), stop=(c == KD - 1))
                    nc.scalar.activation(out=hT[:, f, :], in_=pst[:],
                                         func=AF.Gelu,
                                         bias=b1_sb[:, f:f + 1], scale=1.0)

            # FFN down-projection: two passes of 3 concurrent psum banks,
            # streaming w2 k-tiles (w2 is read twice)
            z2 = act.tile([P, KD, S], BF, name="z", bufs=2)
            for pas in range(2):
                ps_f = [ps.tile([P, S], FP, name=n, bufs=2)
                        for n in ("a", "b", "c")]
                for c in range(KF):
                    w2_sb = wt.tile([P, D], BF, name="w2_sb", bufs=4)
                    nc.sync.dma_start(out=w2_sb[:],
                                      in_=t["w2"][l][c * P:(c + 1) * P, :])
                    for j in range(3):
                        f = pas * 3 + j
                        nc.tensor.matmul(out=ps_f[j][:],
                                         lhsT=w2_sb[:, f * P:(f + 1) * P],
                                         rhs=hT[:, c, :],
                                         start=(c == 0), stop=(c == KF - 1))
                for j in range(3):
                    f = pas * 3 + j
                    nc.vector.tensor_scalar_add(out=ps_f[j][:], in0=ps_f[j][:],
                                                scalar1=b2_sb[:, f:f + 1])
                    nc.vector.tensor_tensor(out=z2[:, f, :], in0=ps_f[j][:],
                                            in1=x1[:, f, :],
                                            op=mybir.AluOpType.add)
            xT = layer_norm(z2, l2g_sb, l2b_sb, "xT")
            if debug_taps:
                dbg = act.tile([P, KD, S], FP, name="dbg", bufs=1)
                for c in range(KD):
                    nc.vector.tensor_copy(out=dbg[:, c, :], in_=xT[:, c, :])
                nc.sync.dma_start(out=taps[f"x{l + 1}"][:], in_=dbg[:])

        # ---- classifier + softmax + compaction ---------------------------
        clsw_sb = const.tile([P, KD, NL], BF)
        nc.sync.dma_start(out=clsw_sb[:],
                          in_=t["clsw"].rearrange("(c p) n -> p c n", p=P))
        clsb_sb = const.tile([NL, 1], FP)
        nc.sync.dma_start(out=clsb_sb[:], in_=t["clsb"][:])
        clsbr_sb = const.tile([1, NL], FP)
        nc.sync.dma_start(out=clsbr_sb[:], in_=t["clsb_row"][:])

        ps_l = ps.tile([NL, S], FP, name="a", bufs=2)
        for c in range(KD):
            nc.tensor.matmul(out=ps_l[:], lhsT=clsw_sb[:, c, :],
                             rhs=xT[:, c, :], start=(c == 0),
                             stop=(c == KD - 1))
        logitsT = misc.tile([NL, S], FP, name="logitsT")
        nc.vector.tensor_scalar_add(out=logitsT[:], in0=ps_l[:],
                                    scalar1=clsb_sb[:])
        if debug_taps:
            nc.sync.dma_start(out=taps["logitsT"][:], in_=logitsT[:])

        # transpose logits to token-major, softmax over the 9 classes
        probs = misc.tile([P, NT, NL], BF, name="probs")
        for ti in range(NT):
            ps_t = ps.tile([P, S], FP, name="b", bufs=2)
            nc.tensor.transpose(out=ps_t[:, :NL],
                                in_=logitsT[:, ti * P:(ti + 1) * P],
                                identity=ident_fp[:NL, :NL])
            ex = misc.tile([P, NL], FP, name="ex", bufs=2)
            den = misc.tile([P, 1], FP, name="den", bufs=2)
            nc.scalar.activation(out=ex[:], in_=ps_t[:, :NL], func=AF.Exp,
                                 accum_out=den[:])
            rden = misc.tile([P, 1], FP, name="rden", bufs=2)
            nc.vector.reciprocal(out=rden[:], in_=den[:])
            nc.vector.tensor_scalar_mul(out=probs[:, ti, :], in0=ex[:],
                                        scalar1=rden[:])
        # padding row: softmax(cls_b)
        exb = misc.tile([1, NL], FP, name="exb")
        denb = misc.tile([1, 1], FP, name="denb")
        nc.scalar.activation(out=exb[:], in_=clsbr_sb[:], func=AF.Exp,
                             accum_out=denb[:])
        rdenb = misc.tile([1, 1], FP, name="rdenb")
        nc.vector.reciprocal(out=rdenb[:], in_=denb[:])
        pad_probs = misc.tile([1, NL], BF, name="pad_probs")
        nc.vector.tensor_scalar_mul(out=pad_probs[:], in0=exb[:],
                                    scalar1=rdenb[:])

        # compaction via permutation matmul
        pmT_sb = wt.tile([P, 5, S], BF, name="bigscratch")
        nc.sync.dma_start(out=pmT_sb[:],
                          in_=t["pmT"].rearrange("(a p) s -> p a s", p=P))
        out_sb = misc.tile([P, NT, NL], FP, name="out_sb")
        for i in range(NT):
            ps_o = ps.tile([P, S], FP, name="c", bufs=2)
            for ti in range(NT):
                nc.tensor.matmul(out=ps_o[:, :NL],
                                 lhsT=pmT_sb[:, ti, i * P:(i + 1) * P],
                                 rhs=probs[:, ti, :],
                                 start=(ti == 0), stop=False)
            nc.tensor.matmul(out=ps_o[:, :NL],
                             lhsT=pmT_sb[0:1, 4, i * P:(i + 1) * P],
                             rhs=pad_probs[:],
                             start=False, stop=True)
            nc.vector.tensor_copy(out=out_sb[:, i, :], in_=ps_o[:, :NL])
        nc.sync.dma_start(out=t["outp"].rearrange("(i p) n -> p i n", p=P),
                          in_=out_sb[:])


_NC_CACHE = {}


def _get_nc(debug_taps=False):
    key = bool(debug_taps)
    if key not in _NC_CACHE:
        _NC_CACHE[key] = build_nc(debug_taps)
    return _NC_CACHE[key]


def make_in_maps(inputs):
    """Build the 8 per-core input maps from the full-problem inputs."""
    inp = {k: np.asarray(v) for k, v in inputs.items()}
    wemb_bf = inp["word_emb"].astype(BF_NP)
    shared = dict(
        wemb=wemb_bf,
        emb_g=inp["emb_g"].reshape(D, 1).astype(np.float32),
        emb_b=inp["emb_b"].reshape(D, 1).astype(np.float32),
        wq=inp["Wq"].astype(BF_NP), wk=inp["Wk"].astype(BF_NP),
        wv=inp["Wv"].astype(BF_NP), wo=inp["Wo"].astype(BF_NP),
        w1=inp["W1"].astype(BF_NP), w2=inp["W2"].astype(BF_NP),
        bq=inp["bq"].reshape(L, D, 1).astype(np.float32),
        bk=inp["bk"].reshape(L, D, 1).astype(np.float32),
        bv=inp["bv"].reshape(L, 1, D).astype(np.float32),
        bo=inp["bo"].reshape(L, D, 1).astype(np.float32),
        b1=inp["b1"].reshape(L, DF, 1).astype(np.float32),
        b2=inp["b2"].reshape(L, D, 1).astype(np.float32),
        ln1g=inp["ln1_g"].reshape(L, D, 1).astype(np.float32),
        ln1b=inp["ln1_b"].reshape(L, D, 1).astype(np.float32),
        ln2g=inp["ln2_g"].reshape(L, D, 1).astype(np.float32),
        ln2b=inp["ln2_b"].reshape(L, D, 1).astype(np.float32),
        clsw=inp["cls_W"].astype(BF_NP),
        clsb=inp["cls_b"].reshape(NL, 1).astype(np.float32),
        clsb_row=inp["cls_b"].reshape(1, NL).astype(np.float32),
    )
    in_maps = []
    for c in range(8):
        b = c // 2
        pt = (inp["pos_emb"] + inp["type_emb"][inp["input_type_ids"][b]])
        posT_b = np.ascontiguousarray(pt.T).astype(BF_NP)
        kbias_b = ((inp["input_mask"][b].astype(np.float32) - 1.0)
                   * 60.0).reshape(S, 1)
        vm = inp["valid_mask"][b]
        order = np.argsort(1 - vm, kind="stable")
        n_valid = int(vm.sum())
        pm = np.zeros((5 * P, S), dtype=BF_NP)
        for i in range(S):
            if i < n_valid:
                pm[order[i], i] = 1
            else:
                pm[S, i] = 1
        in_maps.append(dict(
            shared,
            ids=inp["input_word_ids"][b].reshape(S, 1).astype(np.int32),
            posT=posT_b,
            kbias=kbias_b.astype(np.float32),
            pmT=pm,
        ))
    return in_maps


def kernel(**inputs) -> np.ndarray:
    nc = _get_nc()
    in_maps = make_in_maps(inputs)
    res = run_bass_kernel_spmd(nc, in_maps, core_ids=list(range(8)))
    out = np.stack([res.results[2 * b]["out"] for b in range(B)], axis=0)
    return out.astype(np.float32)


# revision 23
# speedup vs baseline: 1.3577x; 1.3577x over previous
"""Trainium2 Bass kernel for nn_BertNerHF (BERT encoder + NER head with
valid-token stream compaction).

Distribution: data-parallel over the batch (B=4 rows). Each pair of cores
(2b, 2b+1) holds row b; both compute the full row (duplicated pair), outputs
taken from the even cores. No cross-core communication.

On-core layout: activations are kept FEATURE-major in SBUF (xT: [D partitions
(6 tiles of 128), S tokens free]) so that
  - every GEMM is matmul(psum, lhsT=W[kc,kf], rhs=xT[kc, :]) with weights in
    their natural [in,out] layout,
  - per-feature biases / LN gains are per-partition scalars (tensor_scalar),
  - LN token-statistics are computed with ones-column matmuls on the PE and
    re-broadcast across partitions with gpsimd partition_broadcast.
Attention: scoresT (k-major) from lhsT=KT-head, rhs=QT-head; exp fused with
the PSUM eviction on ScalarE (scale=1/8, per-key mask bias); ctx accumulated
as lhsT=V_aug (token-major V with an appended ones column, so row 64 of the
PSUM result is the softmax denominator), then normalized during eviction.
Final compaction is a permutation matmul with a host-built 0/1 matrix (row
512 routes the softmax(cls_b) padding row).

PSUM budget (8 banks, statically reserved per pool tag):
  a(2) b(2) c(2): rotating GEMM/score/ctx/transpose tiles; W2 runs two
  3-bank passes across a/b/c; d(1)/e(1): LN stat rows s1/s2.
"""

import math
from contextlib import ExitStack

import ml_dtypes
import numpy as np

import bass_rust
import concourse.bass as bass
import concourse.mybir as mybir
import concourse.tile as tile
from concourse.bass_utils import run_bass_kernel_spmd

B, S, D, L, H, V, NL = 4, 512, 768, 4, 12, 30522, 9
DH = D // H          # 64
DF = 4 * D           # 3072
P = 128
KD = D // P          # 6  k-tiles over D
KF = DF // P         # 24 k-tiles over DF
NT = S // P          # 4  token tiles
TP = 2               # tensor-parallel degree within a core pair
HL = H // TP         # 6  local heads
DQ = HL * DH         # 384 local qkv width
KQ = DQ // P         # 3  local qkv f-tiles
DFL = DF // TP       # 1536 local FFN width
KFL = DFL // P       # 12 local FFN k-tiles
FP = mybir.dt.float32
BF = mybir.dt.bfloat16
BF_NP = ml_dtypes.bfloat16
AF = mybir.ActivationFunctionType

_MAX_WAITS_PER_INST = 1


def _patched_drain_and_barrier(self, tick_clock, wait_clock):
    """The nix walrus build rejects multi-wait TPB_CTRL (Drain) instructions
    ("Too many sync wait commands"); split the tail drain's waits across
    multiple Drain instructions."""
    from concourse.tile import ScopedClock

    nc = self.nc
    drain_inst = nc.sync.drain()
    wait_clock.add_sem_waits(
        drain_inst.ins, ScopedClock({None: tick_clock.global_clock})
    )
    si = drain_inst.ins.sync_info
    waits = list(si.on_wait or [])
    if len(waits) > _MAX_WAITS_PER_INST:
        drain_inst.ins.sync_info = bass_rust.SyncInfo(
            on_wait=waits[:_MAX_WAITS_PER_INST],
            on_update=list(si.on_update or []),
        )
        for i in range(_MAX_WAITS_PER_INST, len(waits), _MAX_WAITS_PER_INST):
            extra = nc.sync.drain()
            extra.ins.sync_info = bass_rust.SyncInfo(
                on_wait=waits[i : i + _MAX_WAITS_PER_INST], on_update=[]
            )

    nc.all_engine_barrier()
    popped = nc._tile_sem_poison_stack.pop()
    assert popped is self._sem_poison
    nc.clear_and_free_semaphores(list(self.sems.allocated().values()))
    nc.all_engine_barrier()


tile.TileContext._drain_and_barrier = _patched_drain_and_barrier

_MAX_WAITS_GENERIC = 1


def _split_waits(nc, max_waits=_MAX_WAITS_GENERIC):
    """Split multi-wait engine instructions: the nix walrus codegen rejects
    instructions carrying more than a couple of semaphore waits. Excess waits
    move to no-op carrier instructions inserted just before, on the same
    engine. DMA/queue instructions are skipped (their waits are observed by
    the DGE, not the engine sequencer)."""
    # snapshot every block's list BEFORE creating carrier nops (nop() appends
    # to the current block; final reassignment drops those stray copies)
    snaps = [(bb, list(bb.instructions)) for bb in nc.main_func.blocks]

    def needs_split(inst):
        si = inst.sync_info
        return si is not None and len(si.on_wait or []) > max_waits

    new_lists = []
    for bb, insts in snaps:
        new_list = []
        for inst in insts:
            if needs_split(inst):
                si = inst.sync_info
                waits = list(si.on_wait or [])
                excess = waits[:-max_waits]
                eng = nc.engines[inst.engine]
                for j in range(0, len(excess), max_waits):
                    carrier = eng.drain().ins
                    carrier.sync_info = bass_rust.SyncInfo(
                        on_wait=excess[j:j + max_waits], on_update=[])
                    new_list.append(carrier)
                inst.sync_info = bass_rust.SyncInfo(
                    on_wait=waits[-max_waits:],
                    on_update=list(si.on_update or []))
            new_list.append(inst)
        new_lists.append((bb, new_list))
    for bb, new_list in new_lists:
        bb.instructions = new_list


def build_nc(debug_taps=False):
    nc = bass.Bass(trn_type="TRN2", debug=False, num_devices=8)

    # ---- I/O -------------------------------------------------------------
    ios = dict(
        ids=nc.dram_tensor("ids", [S, 1], mybir.dt.int32, kind="ExternalInput"),
        wemb=nc.dram_tensor("wemb", [V, D], BF, kind="ExternalInput"),
        posT=nc.dram_tensor("posT", [D, S], BF, kind="ExternalInput"),
        kbias=nc.dram_tensor("kbias", [S, 1], FP, kind="ExternalInput"),
        emb_g=nc.dram_tensor("emb_g", [D, 1], FP, kind="ExternalInput"),
        emb_b=nc.dram_tensor("emb_b", [D, 1], FP, kind="ExternalInput"),
        wq=nc.dram_tensor("wq", [L, D, DQ], BF, kind="ExternalInput"),
        wk=nc.dram_tensor("wk", [L, D, DQ], BF, kind="ExternalInput"),
        wv=nc.dram_tensor("wv", [L, D, DQ], BF, kind="ExternalInput"),
        wo=nc.dram_tensor("wo", [L, DQ, D], BF, kind="ExternalInput"),
        w1=nc.dram_tensor("w1", [L, D, DFL], BF, kind="ExternalInput"),
        w2=nc.dram_tensor("w2", [L, DFL, D], BF, kind="ExternalInput"),
        bq=nc.dram_tensor("bq", [L, DQ, 1], FP, kind="ExternalInput"),
        bk=nc.dram_tensor("bk", [L, DQ, 1], FP, kind="ExternalInput"),
        bv=nc.dram_tensor("bv", [L, 1, DQ], FP, kind="ExternalInput"),
        bo=nc.dram_tensor("bo", [L, D, 1], FP, kind="ExternalInput"),
        b1=nc.dram_tensor("b1", [L, DFL, 1], FP, kind="ExternalInput"),
        b2=nc.dram_tensor("b2", [L, D, 1], FP, kind="ExternalInput"),
        ln1g=nc.dram_tensor("ln1g", [L, D, 1], FP, kind="ExternalInput"),
        ln1b=nc.dram_tensor("ln1b", [L, D, 1], FP, kind="ExternalInput"),
        ln2g=nc.dram_tensor("ln2g", [L, D, 1], FP, kind="ExternalInput"),
        ln2b=nc.dram_tensor("ln2b", [L, D, 1], FP, kind="ExternalInput"),
        clsw=nc.dram_tensor("clsw", [D, NL], BF, kind="ExternalInput"),
        clsb=nc.dram_tensor("clsb", [NL, 1], FP, kind="ExternalInput"),
        clsb_row=nc.dram_tensor("clsb_row", [1, NL], FP, kind="ExternalInput"),
        pmT=nc.dram_tensor("pmT", [5 * P, S], BF, kind="ExternalInput"),
        outp=nc.dram_tensor("out", [S, NL], FP, kind="ExternalOutput"),
    )
    ios["arbufs"] = [
        tuple(nc.dram_tensor(f"ar{j}_{l}", [D, S], BF)
              for j in range(4))
        for l in range(L)
    ]
    taps = {}
    if debug_taps:
        for nm in ["x0", "x1", "x2", "x3", "x4"]:
            taps[nm] = nc.dram_tensor("tap_" + nm, [P, KD, S], FP,
                                      kind="ExternalOutput")
        taps["logitsT"] = nc.dram_tensor("tap_logitsT", [NL, S], FP,
                                         kind="ExternalOutput")

    with tile.TileContext(nc) as tc:
        _build_body(nc, tc, ios, debug_taps, taps)
    _split_waits(nc)
    return nc


def _build_body(nc, tc, t, debug_taps, taps):
    with ExitStack() as ctx:
        const = ctx.enter_context(tc.tile_pool(name="const", bufs=1))
        act = ctx.enter_context(tc.tile_pool(name="act", bufs=1))
        wt = ctx.enter_context(tc.tile_pool(name="wt", bufs=1))
        misc = ctx.enter_context(tc.tile_pool(name="misc", bufs=1))
        ps = ctx.enter_context(tc.tile_pool(name="ps", bufs=1, space="PSUM"))

        # ---- constants ---------------------------------------------------
        ident_bf = const.tile([P, P], BF)
        from concourse.masks import make_identity
        make_identity(nc, ident_bf[:])
        ident_fp = const.tile([P, P], FP)
        make_identity(nc, ident_fp[:])
        ones_col = const.tile([P, 1], BF)
        nc.vector.memset(ones_col[:], 1.0)
        ones_row = const.tile([1, P], FP)
        nc.vector.memset(ones_row[:], 1.0)
        eps_t = const.tile([1, 1], FP)
        nc.vector.memset(eps_t[:], 1e-12)

        posT_sb = wt.tile([P, KD, S], BF, name="bigscratch")
        nc.sync.dma_start(out=posT_sb[:],
                          in_=t["posT"].rearrange("(c p) s -> p c s", p=P))
        kb_sb = const.tile([P, NT], FP)
        nc.sync.dma_start(out=kb_sb[:],
                          in_=t["kbias"].rearrange("(t p) 1 -> p t", p=P))
        ids_sb = const.tile([P, NT], mybir.dt.int32)
        nc.sync.dma_start(out=ids_sb[:],
                          in_=t["ids"].rearrange("(t p) 1 -> p t", p=P))
        embg_sb = const.tile([P, KD], FP)
        nc.sync.dma_start(out=embg_sb[:],
                          in_=t["emb_g"].rearrange("(c p) 1 -> p c", p=P))
        embb_sb = const.tile([P, KD], FP)
        nc.sync.dma_start(out=embb_sb[:],
                          in_=t["emb_b"].rearrange("(c p) 1 -> p c", p=P))

        # ---- embedding: gather + transpose + pos/type + LN ---------------
        we_t = []
        for ti in range(NT):
            g = misc.tile([P, D], BF, name="wegather", bufs=4)
            nc.gpsimd.indirect_dma_start(
                out=g[:], out_offset=None, in_=t["wemb"][:, :],
                in_offset=bass.IndirectOffsetOnAxis(ap=ids_sb[:, ti:ti + 1],
                                                    axis=0))
            we_t.append(g)
        z = act.tile([P, KD, S], BF, name="z", bufs=2)
        for f in range(KD):
            pst = ps.tile([P, S], BF, name="a", bufs=2)
            for ti in range(NT):
                nc.tensor.transpose(out=pst[:, ti * P:(ti + 1) * P],
                                    in_=we_t[ti][:, f * P:(f + 1) * P],
                                    identity=ident_bf[:])
            nc.vector.tensor_add(out=z[:, f, :], in0=pst[:],
                                 in1=posT_sb[:, f, :])

        def layer_norm(zt, g_sb, b_sb, name):
            """LN over the feature (partition) dim of zt [P, KD, S] -> bf16."""
            s1 = ps.tile([1, S], FP, name="d", bufs=1)
            s2 = ps.tile([1, S], FP, name="e", bufs=1)
            for c in range(KD):
                nc.tensor.matmul(out=s1[:], lhsT=ones_col[:], rhs=zt[:, c, :],
                                 start=(c == 0), stop=(c == KD - 1))
                sq = misc.tile([P, S], BF, name="sqs", bufs=2)
                nc.vector.tensor_mul(out=sq[:], in0=zt[:, c, :],
                                     in1=zt[:, c, :])
                nc.tensor.matmul(out=s2[:], lhsT=ones_col[:], rhs=sq[:],
                                 start=(c == 0), stop=(c == KD - 1))
            m2 = misc.tile([1, S], FP, name="m2")
            nc.scalar.activation(out=m2[:], in_=s1[:], func=AF.Square,
                                 scale=1.0 / math.sqrt(D))
            u = misc.tile([1, S], FP, name="u")
            nc.vector.tensor_tensor(out=u[:], in0=s2[:], in1=m2[:],
                                    op=mybir.AluOpType.subtract)
            r = misc.tile([1, S], FP, name="r")
            nc.scalar.activation(out=r[:], in_=u[:], func=AF.Sqrt,
                                 bias=eps_t[:], scale=1.0 / D)
            rstd = misc.tile([1, S], FP, name="rstd")
            nc.vector.reciprocal(out=rstd[:], in_=r[:])
            mu = misc.tile([1, S], FP, name="mu")
            nc.scalar.mul(out=mu[:], in_=s1[:], mul=1.0 / D)
            mu_b = ps.tile([P, S], FP, name="d", bufs=1)
            nc.tensor.matmul(out=mu_b[:], lhsT=ones_row[:], rhs=mu[:],
                             start=True, stop=True)
            rstd_b = ps.tile([P, S], FP, name="e", bufs=1)
            nc.tensor.matmul(out=rstd_b[:], lhsT=ones_row[:], rhs=rstd[:],
                             start=True, stop=True)
            xo = act.tile([P, KD, S], BF, name=name, bufs=2)
            for c in range(KD):
                tt = misc.tile([P, S], FP, name="lnt", bufs=2)
                nc.vector.tensor_tensor(out=tt[:], in0=zt[:, c, :],
                                        in1=mu_b[:],
                                        op=mybir.AluOpType.subtract)
                nc.vector.tensor_mul(out=tt[:], in0=tt[:], in1=rstd_b[:])
                nc.vector.tensor_scalar(
                    out=xo[:, c, :], in0=tt[:],
                    scalar1=g_sb[:, c:c + 1], scalar2=b_sb[:, c:c + 1],
                    op0=mybir.AluOpType.mult, op1=mybir.AluOpType.add)
            return xo

        xT = layer_norm(z, embg_sb, embb_sb, "xT")
        if debug_taps:
            dbg = act.tile([P, KD, S], FP, name="dbg", bufs=1)
            for c in range(KD):
                nc.vector.tensor_copy(out=dbg[:, c, :], in_=xT[:, c, :])
            nc.sync.dma_start(out=taps["x0"][:], in_=dbg[:])

        def col_bias(src, ncols, name):
            b_sb = misc.tile([P, ncols], FP, name=name)
            nc.sync.dma_start(out=b_sb[:],
                              in_=src.rearrange("(c p) 1 -> p c", p=P))
            return b_sb

        # ---- transformer layers ------------------------------------------
        for l in range(L):
            bq_sb = col_bias(t["bq"][l], KQ, "bq_sb")
            bk_sb = col_bias(t["bk"][l], KQ, "bk_sb")
            bo_sb = col_bias(t["bo"][l], KD, "bo_sb")
            b2_sb = col_bias(t["b2"][l], KD, "b2_sb")
            b1_sb = col_bias(t["b1"][l], KFL, "b1_sb")
            l1g_sb = col_bias(t["ln1g"][l], KD, "l1g_sb")
            l1b_sb = col_bias(t["ln1b"][l], KD, "l1b_sb")
            l2g_sb = col_bias(t["ln2g"][l], KD, "l2g_sb")
            l2b_sb = col_bias(t["ln2b"][l], KD, "l2b_sb")
            bv_row = misc.tile([1, DQ], FP, name="bv_row")
            nc.sync.dma_start(out=bv_row[:], in_=t["bv"][l])

            # Q/K projections -> feature-major QT/KT [P, KD, S]
            wq_sb = wt.tile([P, KD, DQ], BF, name="wx", bufs=2)
            nc.sync.dma_start(out=wq_sb[:],
                              in_=t["wq"][l].rearrange("(c p) f -> p c f", p=P))
            wk_sb = wt.tile([P, KD, DQ], BF, name="wx", bufs=2)
            nc.sync.dma_start(out=wk_sb[:],
                              in_=t["wk"][l].rearrange("(c p) f -> p c f", p=P))
            qT = act.tile([P, KQ, S], BF, name="qT")
            kT = act.tile([P, KQ, S], BF, name="kT")
            for dst, w_sb, b_sb in ((qT, wq_sb, bq_sb), (kT, wk_sb, bk_sb)):
                for f in range(KQ):
                    pst = ps.tile([P, S], FP, name="a", bufs=2)
                    for c in range(KD):
                        nc.tensor.matmul(out=pst[:],
                                         lhsT=w_sb[:, c, f * P:(f + 1) * P],
                                         rhs=xT[:, c, :],
                                         start=(c == 0), stop=(c == KD - 1))
                    nc.vector.tensor_scalar_add(out=dst[:, f, :], in0=pst[:],
                                                scalar1=b_sb[:, f:f + 1])

            # V projection -> token-major V_aug [P(tok), NT, H, DH+1]
            wv_sb = wt.tile([P, KD, DQ], BF, name="wx", bufs=2)
            nc.sync.dma_start(out=wv_sb[:],
                              in_=t["wv"][l].rearrange("(c p) f -> p c f", p=P))
            va = act.tile([P, NT, HL, DH + 1], BF, name="va")
            nc.vector.memset(va[:, :, :, DH:DH + 1], 1.0)
            for ti in range(NT):
                for fb in range(1):
                    pst = ps.tile([P, 384], FP, name="b", bufs=2)
                    nc.tensor.matmul(
                        out=pst[:],
                        lhsT=ones_row[:],
                        rhs=bv_row[:, fb * 384:(fb + 1) * 384],
                        start=True, stop=False)
                    for c in range(KD):
                        nc.tensor.matmul(
                            out=pst[:],
                            lhsT=xT[:, c, ti * P:(ti + 1) * P],
                            rhs=wv_sb[:, c, fb * 384:(fb + 1) * 384],
                            start=False, stop=(c == KD - 1))
                    nc.vector.tensor_copy(
                        out=va[:, ti, fb * 6:(fb + 1) * 6, 0:DH],
                        in_=pst.rearrange("p (a b) -> p a b", a=6))

            # attention per head
            ctxT = act.tile([P, KQ, S], BF, name="ctxT")
            for h in range(HL):
                hp = (h % 2) * DH
                hf = h // 2
                expT = act.tile([P, NT, S], BF, name="expT", bufs=2)
                for kt in range(NT):
                    ps_s = ps.tile([P, S], FP, name="b", bufs=2)
                    nc.tensor.matmul(
                        out=ps_s[:],
                        lhsT=kT[hp:hp + DH, hf, kt * P:(kt + 1) * P],
                        rhs=qT[hp:hp + DH, hf, :],
                        start=True, stop=True)
                    nc.scalar.activation(
                        out=expT[:, kt, :], in_=ps_s[:], func=AF.Exp,
                        scale=1.0 / math.sqrt(DH),
                        bias=kb_sb[:, kt:kt + 1])
                ps_c = ps.tile([P, S], FP, name="c", bufs=2)
                for kt in range(NT):
                    nc.tensor.matmul(out=ps_c[:DH + 1, :],
                                     lhsT=va[:, kt, h, :],
                                     rhs=expT[:, kt, :],
                                     start=(kt == 0), stop=(kt == NT - 1))
                rec = misc.tile([1, S], FP, name="rec", bufs=2)
                nc.vector.reciprocal(out=rec[:], in_=ps_c[DH:DH + 1, :])
                rec_b = ps.tile([P, S], FP, name="b", bufs=2)
                nc.tensor.matmul(out=rec_b[:DH, :],
                                 lhsT=ones_row[:, :DH], rhs=rec[:],
                                 start=True, stop=True)
                craw = misc.tile([DH, S], FP, name="craw", bufs=2)
                nc.vector.tensor_copy(out=craw[:], in_=ps_c[:DH, :])
                nc.vector.tensor_tensor(out=ctxT[hp:hp + DH, hf, :],
                                        in0=craw[:], in1=rec_b[:DH, :],
                                        op=mybir.AluOpType.mult)

            # attention output projection + residual + LN1
            wo_sb = wt.tile([P, KQ, D], BF, name="wx", bufs=2)
            nc.sync.dma_start(out=wo_sb[:],
                              in_=t["wo"][l].rearrange("(c p) f -> p c f", p=P))
            zp = act.tile([P, KD, S], BF, name="zp", bufs=2)
            for f in range(KD):
                pst = ps.tile([P, S], FP, name="a", bufs=2)
                for c in range(KQ):
                    nc.tensor.matmul(out=pst[:],
                                     lhsT=wo_sb[:, c, f * P:(f + 1) * P],
                                     rhs=ctxT[:, c, :],
                                     start=(c == 0), stop=(c == KQ - 1))
                nc.vector.tensor_scalar_add(out=zp[:, f, :], in0=pst[:],
                                            scalar1=bo_sb[:, f:f + 1])
            ar_i, ar_o = t["arbufs"][l][0], t["arbufs"][l][1]
            nc.sync.dma_start(out=ar_i.rearrange("(c p) s -> p c s", p=P),
                              in_=zp[:])
            nc.gpsimd.collective_compute(
                "AllReduce", mybir.AluOpType.add,
                replica_groups=[[0, 1], [2, 3], [4, 5], [6, 7]],
                ins=[ar_i[:]], outs=[ar_o[:]])
            yT = act.tile([P, KD, S], BF, name="zp", bufs=2)
            nc.sync.dma_start(out=yT[:],
                              in_=ar_o.rearrange("(c p) s -> p c s", p=P))
            z1 = act.tile([P, KD, S], BF, name="z", bufs=2)
            for f in range(KD):
                nc.vector.tensor_tensor(out=z1[:, f, :], in0=yT[:, f, :],
                                        in1=xT[:, f, :],
                                        op=mybir.AluOpType.add)
            x1 = layer_norm(z1, l1g_sb, l1b_sb, "xT")

            # FFN up-projection + gelu, W1 streamed in two halves
            hT = act.tile([P, KFL, S], BF, name="hT")
            for half in range(2):
                w1_sb = wt.tile([P, KD, DFL // 2], BF, name="w1h", bufs=1)
                nc.sync.dma_start(
                    out=w1_sb[:],
                    in_=t["w1"][l].rearrange("(c p) f -> p c f", p=P)[
                        :, :, half * (DFL // 2):(half + 1) * (DFL // 2)])
                for fi in range(KFL // 2):
                    f = half * (KFL // 2) + fi
                    pst = ps.tile([P, S], FP, name="a", bufs=2)
                    for c in range(KD):
                        nc.tensor.matmul(out=pst[:],
                                         lhsT=w1_sb[:, c, fi * P:(fi + 1) * P],
                                         rhs=x1[:, c, :],
                                         start=(c == 0), stop=(c == KD - 1))
                    nc.scalar.activation(out=hT[:, f, :], in_=pst[:],
                                         func=AF.Gelu,
                                         bias=b1_sb[:, f:f + 1], scale=1.0)

            # FFN down-projection: two passes of 3 concurrent psum banks,
            # streaming w2 k-tiles (w2 is read twice)
            zp2 = act.tile([P, KD, S], BF, name="zp", bufs=2)
            for pas in range(2):
                ps_f = [ps.tile([P, S], FP, name=n, bufs=2)
                        for n in ("a", "b", "c")]
                for c in range(KFL):
                    w2_sb = wt.tile([P, D], BF, name="w2_sb", bufs=4)
                    nc.sync.dma_start(out=w2_sb[:],
                                      in_=t["w2"][l][c * P:(c + 1) * P, :])
                    for j in range(3):
                        f = pas * 3 + j
                        nc.tensor.matmul(out=ps_f[j][:],
                                         lhsT=w2_sb[:, f * P:(f + 1) * P],
                                         rhs=hT[:, c, :],
                                         start=(c == 0), stop=(c == KFL - 1))
                for j in range(3):
                    f = pas * 3 + j
                    nc.vector.tensor_scalar_add(out=zp2[:, f, :],
                                                in0=ps_f[j][:],
                                                scalar1=b2_sb[:, f:f + 1])
            ar2_i, ar2_o = t["arbufs"][l][2], t["arbufs"][l][3]
            nc.sync.dma_start(out=ar2_i.rearrange("(c p) s -> p c s", p=P),
                              in_=zp2[:])
            nc.gpsimd.collective_compute(
                "AllReduce", mybir.AluOpType.add,
                replica_groups=[[0, 1], [2, 3], [4, 5], [6, 7]],
                ins=[ar2_i[:]], outs=[ar2_o[:]])
            yT2 = act.tile([P, KD, S], BF, name="zp", bufs=2)
            nc.sync.dma_start(out=yT2[:],
                              in_=ar2_o.rearrange("(c p) s -> p c s", p=P))
            z2 = act.tile([P, KD, S], BF, name="z", bufs=2)
            for f in range(KD):
                nc.vector.tensor_tensor(out=z2[:, f, :], in0=yT2[:, f, :],
                                        in1=x1[:, f, :],
                                        op=mybir.AluOpType.add)
            xT = layer_norm(z2, l2g_sb, l2b_sb, "xT")
            if debug_taps:
                dbg = act.tile([P, KD, S], FP, name="dbg", bufs=1)
                for c in range(KD):
                    nc.vector.tensor_copy(out=dbg[:, c, :], in_=xT[:, c, :])
                nc.sync.dma_start(out=taps[f"x{l + 1}"][:], in_=dbg[:])

        # ---- classifier + softmax + compaction ---------------------------
        clsw_sb = const.tile([P, KD, NL], BF)
        nc.sync.dma_start(out=clsw_sb[:],
                          in_=t["clsw"].rearrange("(c p) n -> p c n", p=P))
        clsb_sb = const.tile([NL, 1], FP)
        nc.sync.dma_start(out=clsb_sb[:], in_=t["clsb"][:])
        clsbr_sb = const.tile([1, NL], FP)
        nc.sync.dma_start(out=clsbr_sb[:], in_=t["clsb_row"][:])

        ps_l = ps.tile([NL, S], FP, name="a", bufs=2)
        for c in range(KD):
            nc.tensor.matmul(out=ps_l[:], lhsT=clsw_sb[:, c, :],
                             rhs=xT[:, c, :], start=(c == 0),
                             stop=(c == KD - 1))
        logitsT = misc.tile([NL, S], FP, name="logitsT")
        nc.vector.tensor_scalar_add(out=logitsT[:], in0=ps_l[:],
                                    scalar1=clsb_sb[:])
        if debug_taps:
            nc.sync.dma_start(out=taps["logitsT"][:], in_=logitsT[:])

        # transpose logits to token-major, softmax over the 9 classes
        probs = misc.tile([P, NT, NL], BF, name="probs")
        for ti in range(NT):
            ps_t = ps.tile([P, S], FP, name="b", bufs=2)
            nc.tensor.transpose(out=ps_t[:, :NL],
                                in_=logitsT[:, ti * P:(ti + 1) * P],
                                identity=ident_fp[:NL, :NL])
            ex = misc.tile([P, NL], FP, name="ex", bufs=2)
            den = misc.tile([P, 1], FP, name="den", bufs=2)
            nc.scalar.activation(out=ex[:], in_=ps_t[:, :NL], func=AF.Exp,
                                 accum_out=den[:])
            rden = misc.tile([P, 1], FP, name="rden", bufs=2)
            nc.vector.reciprocal(out=rden[:], in_=den[:])
            nc.vector.tensor_scalar_mul(out=probs[:, ti, :], in0=ex[:],
                                        scalar1=rden[:])
        # padding row: softmax(cls_b)
        exb = misc.tile([1, NL], FP, name="exb")
        denb = misc.tile([1, 1], FP, name="denb")
        nc.scalar.activation(out=exb[:], in_=clsbr_sb[:], func=AF.Exp,
                             accum_out=denb[:])
        rdenb = misc.tile([1, 1], FP, name="rdenb")
        nc.vector.reciprocal(out=rdenb[:], in_=denb[:])
        pad_probs = misc.tile([1, NL], BF, name="pad_probs")
        nc.vector.tensor_scalar_mul(out=pad_probs[:], in0=exb[:],
                                    scalar1=rdenb[:])

        # compaction via permutation matmul
        pmT_sb = wt.tile([P, 5, S], BF, name="bigscratch")
        nc.sync.dma_start(out=pmT_sb[:],
                          in_=t["pmT"].rearrange("(a p) s -> p a s", p=P))
        out_sb = misc.tile([P, NT, NL], FP, name="out_sb")
        for i in range(NT):
            ps_o = ps.tile([P, S], FP, name="c", bufs=2)
            for ti in range(NT):
                nc.tensor.matmul(out=ps_o[:, :NL],
                                 lhsT=pmT_sb[:, ti, i * P:(i + 1) * P],
                                 rhs=probs[:, ti, :],
                                 start=(ti == 0), stop=False)
            nc.tensor.matmul(out=ps_o[:, :NL],
                             lhsT=pmT_sb[0:1, 4, i * P:(i + 1) * P],
                             rhs=pad_probs[:],
                             start=False, stop=True)
            nc.vector.tensor_copy(out=out_sb[:, i, :], in_=ps_o[:, :NL])
        nc.sync.dma_start(out=t["outp"].rearrange("(i p) n -> p i n", p=P),
                          in_=out_sb[:])


_NC_CACHE = {}


def _get_nc(debug_taps=False):
    key = bool(debug_taps)
    if key not in _NC_CACHE:
        _NC_CACHE[key] = build_nc(debug_taps)
    return _NC_CACHE[key]


def make_in_maps(inputs):
    """Build the 8 per-core input maps from the full-problem inputs."""
    inp = {k: np.asarray(v) for k, v in inputs.items()}
    wemb_bf = inp["word_emb"].astype(BF_NP)
    shared = dict(
        wemb=wemb_bf,
        emb_g=inp["emb_g"].reshape(D, 1).astype(np.float32),
        emb_b=inp["emb_b"].reshape(D, 1).astype(np.float32),
        ln1g=inp["ln1_g"].reshape(L, D, 1).astype(np.float32),
        ln1b=inp["ln1_b"].reshape(L, D, 1).astype(np.float32),
        ln2g=inp["ln2_g"].reshape(L, D, 1).astype(np.float32),
        ln2b=inp["ln2_b"].reshape(L, D, 1).astype(np.float32),
        clsw=inp["cls_W"].astype(BF_NP),
        clsb=inp["cls_b"].reshape(NL, 1).astype(np.float32),
        clsb_row=inp["cls_b"].reshape(1, NL).astype(np.float32),
    )
    shards = []
    for s_ in range(TP):
        qs = slice(s_ * DQ, (s_ + 1) * DQ)
        fs = slice(s_ * DFL, (s_ + 1) * DFL)
        shards.append(dict(
            wq=np.ascontiguousarray(inp["Wq"][:, :, qs]).astype(BF_NP),
            wk=np.ascontiguousarray(inp["Wk"][:, :, qs]).astype(BF_NP),
            wv=np.ascontiguousarray(inp["Wv"][:, :, qs]).astype(BF_NP),
            wo=np.ascontiguousarray(inp["Wo"][:, qs, :]).astype(BF_NP),
            w1=np.ascontiguousarray(inp["W1"][:, :, fs]).astype(BF_NP),
            w2=np.ascontiguousarray(inp["W2"][:, fs, :]).astype(BF_NP),
            bq=np.ascontiguousarray(
                inp["bq"][:, qs]).reshape(L, DQ, 1).astype(np.float32),
            bk=np.ascontiguousarray(
                inp["bk"][:, qs]).reshape(L, DQ, 1).astype(np.float32),
            bv=np.ascontiguousarray(
                inp["bv"][:, qs]).reshape(L, 1, DQ).astype(np.float32),
            bo=(inp["bo"] / TP).reshape(L, D, 1).astype(np.float32),
            b1=np.ascontiguousarray(
                inp["b1"][:, fs]).reshape(L, DFL, 1).astype(np.float32),
            b2=(inp["b2"] / TP).reshape(L, D, 1).astype(np.float32),
        ))
    in_maps = []
    for c in range(8):
        b = c // 2
        pt = (inp["pos_emb"] + inp["type_emb"][inp["input_type_ids"][b]])
        posT_b = np.ascontiguousarray(pt.T).astype(BF_NP)
        kbias_b = ((inp["input_mask"][b].astype(np.float32) - 1.0)
                   * 60.0).reshape(S, 1)
        vm = inp["valid_mask"][b]
        order = np.argsort(1 - vm, kind="stable")
        n_valid = int(vm.sum())
        pm = np.zeros((5 * P, S), dtype=BF_NP)
        for i in range(S):
            if i < n_valid:
                pm[order[i], i] = 1
            else:
                pm[S, i] = 1
        in_maps.append(dict(
            shared, **shards[c % 2],
            ids=inp["input_word_ids"][b].reshape(S, 1).astype(np.int32),
            posT=posT_b,
            kbias=kbias_b.astype(np.float32),
            pmT=pm,
        ))
    return in_maps


def kernel(**inputs) -> np.ndarray:
    nc = _get_nc()
    in_maps = make_in_maps(inputs)
    res = run_bass_kernel_spmd(nc, in_maps, core_ids=list(range(8)))
    out = np.stack([res.results[2 * b]["out"] for b in range(B)], axis=0)
    return out.astype(np.float32)


# revision 42
# speedup vs baseline: 2.7049x; 1.9923x over previous
"""Trainium2 Bass kernel for nn_BertNerHF (BERT encoder + NER head with
valid-token stream compaction).

Distribution: data-parallel over the batch (B=4 rows). Each pair of cores
(2b, 2b+1) holds row b; both compute the full row (duplicated pair), outputs
taken from the even cores. No cross-core communication.

On-core layout: activations are kept FEATURE-major in SBUF (xT: [D partitions
(6 tiles of 128), S tokens free]) so that
  - every GEMM is matmul(psum, lhsT=W[kc,kf], rhs=xT[kc, :]) with weights in
    their natural [in,out] layout,
  - per-feature biases / LN gains are per-partition scalars (tensor_scalar),
  - LN token-statistics are computed with ones-column matmuls on the PE and
    re-broadcast across partitions with K=1 ones-row matmuls into PSUM.
Attention: scoresT (k-major) from lhsT=KT-head, rhs=QT-head; exp fused with
the PSUM eviction on ScalarE (scale=1/8, per-key mask bias); ctx accumulated
as lhsT=V_aug (token-major V with an appended ones column, so row 64 of the
PSUM result is the softmax denominator), then normalized during eviction.
Final compaction is a permutation matmul with a host-built 0/1 matrix (row
512 routes the softmax(cls_b) padding row).

PSUM budget (8 banks, statically reserved per pool tag):
  a(2) b(2) c(2): rotating GEMM/score/ctx/transpose tiles; W2 runs two
  3-bank passes across a/b/c; d(1)/e(1): LN stat rows s1/s2.
"""

import math
from contextlib import ExitStack

import ml_dtypes
import numpy as np

import bass_rust
import concourse.bass as bass
import concourse.mybir as mybir
import concourse.tile as tile
from concourse.bass_utils import run_bass_kernel_spmd

B, S, D, L, H, V, NL = 4, 512, 768, 4, 12, 30522, 9
DH = D // H          # 64
DF = 4 * D           # 3072
P = 128
KD = D // P          # 6  k-tiles over D
KF = DF // P         # 24 k-tiles over DF
NT = S // P          # 4  token tiles
FP = mybir.dt.float32
BF = mybir.dt.bfloat16
BF_NP = ml_dtypes.bfloat16
AF = mybir.ActivationFunctionType

_MAX_WAITS_PER_INST = 1


def _patched_drain_and_barrier(self, tick_clock, wait_clock):
    """The nix walrus build rejects multi-wait TPB_CTRL (Drain) instructions
    ("Too many sync wait commands"); split the tail drain's waits across
    multiple Drain instructions."""
    from concourse.tile import ScopedClock

    nc = self.nc
    drain_inst = nc.sync.drain()
    wait_clock.add_sem_waits(
        drain_inst.ins, ScopedClock({None: tick_clock.global_clock})
    )
    si = drain_inst.ins.sync_info
    waits = list(si.on_wait or [])
    if len(waits) > _MAX_WAITS_PER_INST:
        drain_inst.ins.sync_info = bass_rust.SyncInfo(
            on_wait=waits[:_MAX_WAITS_PER_INST],
            on_update=list(si.on_update or []),
        )
        for i in range(_MAX_WAITS_PER_INST, len(waits), _MAX_WAITS_PER_INST):
            extra = nc.sync.drain()
            extra.ins.sync_info = bass_rust.SyncInfo(
                on_wait=waits[i : i + _MAX_WAITS_PER_INST], on_update=[]
            )

    nc.all_engine_barrier()
    popped = nc._tile_sem_poison_stack.pop()
    assert popped is self._sem_poison
    nc.clear_and_free_semaphores(list(self.sems.allocated().values()))
    nc.all_engine_barrier()


tile.TileContext._drain_and_barrier = _patched_drain_and_barrier

_MAX_WAITS_GENERIC = 1


def _split_waits(nc, max_waits=_MAX_WAITS_GENERIC):
    """Split multi-wait instructions: the nix walrus codegen rejects
    instructions carrying more than one semaphore wait. Excess waits move to
    Drain carrier instructions inserted just before, on the same engine."""
    # snapshot every block's list BEFORE creating carrier nops (nop() appends
    # to the current block; final reassignment drops those stray copies)
    snaps = [(bb, list(bb.instructions)) for bb in nc.main_func.blocks]

    def needs_split(inst):
        si = inst.sync_info
        return si is not None and len(si.on_wait or []) > max_waits

    new_lists = []
    for bb, insts in snaps:
        new_list = []
        for inst in insts:
            if needs_split(inst):
                si = inst.sync_info
                waits = list(si.on_wait or [])
                excess = waits[:-max_waits]
                eng = nc.engines[inst.engine]
                for j in range(0, len(excess), max_waits):
                    carrier = eng.nop().ins
                    carrier.sync_info = bass_rust.SyncInfo(
                        on_wait=excess[j:j + max_waits], on_update=[])
                    new_list.append(carrier)
                inst.sync_info = bass_rust.SyncInfo(
                    on_wait=waits[-max_waits:],
                    on_update=list(si.on_update or []))
            new_list.append(inst)
        new_lists.append((bb, new_list))
    for bb, new_list in new_lists:
        bb.instructions = new_list


def build_nc(debug_taps=False):
    nc = bass.Bass(trn_type="TRN2", debug=False, num_devices=8)

    # ---- I/O -------------------------------------------------------------
    ios = dict(
        ids=nc.dram_tensor("ids", [S, 1], mybir.dt.int32, kind="ExternalInput"),
        wemb=nc.dram_tensor("wemb", [V, D], BF, kind="ExternalInput"),
        posT=nc.dram_tensor("posT", [D, S], BF, kind="ExternalInput"),
        kbias=nc.dram_tensor("kbias", [S, 1], FP, kind="ExternalInput"),
        emb_g=nc.dram_tensor("emb_g", [D, 1], FP, kind="ExternalInput"),
        emb_b=nc.dram_tensor("emb_b", [D, 1], FP, kind="ExternalInput"),
        wq=nc.dram_tensor("wq", [L, D, D], BF, kind="ExternalInput"),
        wk=nc.dram_tensor("wk", [L, D, D], BF, kind="ExternalInput"),
        wv=nc.dram_tensor("wv", [L, D, D], BF, kind="ExternalInput"),
        wo=nc.dram_tensor("wo", [L, D, D], BF, kind="ExternalInput"),
        w1=nc.dram_tensor("w1", [L, D, DF], BF, kind="ExternalInput"),
        w2=nc.dram_tensor("w2", [L, DF, D], BF, kind="ExternalInput"),
        bq=nc.dram_tensor("bq", [L, D, 1], FP, kind="ExternalInput"),
        bk=nc.dram_tensor("bk", [L, D, 1], FP, kind="ExternalInput"),
        bv=nc.dram_tensor("bv", [L, 1, D], FP, kind="ExternalInput"),
        bo=nc.dram_tensor("bo", [L, D, 1], FP, kind="ExternalInput"),
        b1=nc.dram_tensor("b1", [L, DF, 1], FP, kind="ExternalInput"),
        b2=nc.dram_tensor("b2", [L, D, 1], FP, kind="ExternalInput"),
        ln1g=nc.dram_tensor("ln1g", [L, D, 1], FP, kind="ExternalInput"),
        ln1b=nc.dram_tensor("ln1b", [L, D, 1], FP, kind="ExternalInput"),
        ln2g=nc.dram_tensor("ln2g", [L, D, 1], FP, kind="ExternalInput"),
        ln2b=nc.dram_tensor("ln2b", [L, D, 1], FP, kind="ExternalInput"),
        clsw=nc.dram_tensor("clsw", [D, NL], BF, kind="ExternalInput"),
        clsb=nc.dram_tensor("clsb", [NL, 1], FP, kind="ExternalInput"),
        clsb_row=nc.dram_tensor("clsb_row", [1, NL], FP, kind="ExternalInput"),
        pmT=nc.dram_tensor("pmT", [5 * P, S], BF, kind="ExternalInput"),
        outp=nc.dram_tensor("out", [S, NL], FP, kind="ExternalOutput"),
    )
    taps = {}
    if debug_taps:
        for nm in ["x0", "x1", "x2", "x3", "x4"]:
            taps[nm] = nc.dram_tensor("tap_" + nm, [P, KD, S], FP,
                                      kind="ExternalOutput")
        taps["logitsT"] = nc.dram_tensor("tap_logitsT", [NL, S], FP,
                                         kind="ExternalOutput")

    with tile.TileContext(nc) as tc:
        _build_body(nc, tc, ios, debug_taps, taps)
    _split_waits(nc)
    return nc


def _build_body(nc, tc, t, debug_taps, taps):
    with ExitStack() as ctx:
        const = ctx.enter_context(tc.tile_pool(name="const", bufs=1))
        act = ctx.enter_context(tc.tile_pool(name="act", bufs=1))
        wt = ctx.enter_context(tc.tile_pool(name="wt", bufs=1))
        misc = ctx.enter_context(tc.tile_pool(name="misc", bufs=1))
        ps = ctx.enter_context(tc.tile_pool(name="ps", bufs=1, space="PSUM"))

        # ---- constants ---------------------------------------------------
        ident_bf = const.tile([P, P], BF)
        from concourse.masks import make_identity
        make_identity(nc, ident_bf[:])
        ident_fp = const.tile([P, P], FP)
        make_identity(nc, ident_fp[:])
        ones_col = const.tile([P, 1], BF)
        nc.vector.memset(ones_col[:], 1.0)
        ones_row = const.tile([1, P], FP)
        nc.vector.memset(ones_row[:], 1.0)
        eps_t = const.tile([1, 1], FP)
        nc.vector.memset(eps_t[:], 1e-12)

        posT_sb = wt.tile([P, KD, S], BF, name="bigscratch")
        nc.sync.dma_start(out=posT_sb[:],
                          in_=t["posT"].rearrange("(c p) s -> p c s", p=P))
        kb_sb = const.tile([P, NT], FP)
        nc.sync.dma_start(out=kb_sb[:],
                          in_=t["kbias"].rearrange("(t p) 1 -> p t", p=P))
        ids_sb = const.tile([P, NT], mybir.dt.int32)
        nc.sync.dma_start(out=ids_sb[:],
                          in_=t["ids"].rearrange("(t p) 1 -> p t", p=P))
        embg_sb = const.tile([P, KD], FP)
        nc.sync.dma_start(out=embg_sb[:],
                          in_=t["emb_g"].rearrange("(c p) 1 -> p c", p=P))
        embb_sb = const.tile([P, KD], FP)
        nc.sync.dma_start(out=embb_sb[:],
                          in_=t["emb_b"].rearrange("(c p) 1 -> p c", p=P))

        # ---- embedding: gather + transpose + pos/type + LN ---------------
        we_t = []
        for ti in range(NT):
            g = misc.tile([P, D], BF, name="wegather", bufs=4)
            nc.gpsimd.indirect_dma_start(
                out=g[:], out_offset=None, in_=t["wemb"][:, :],
                in_offset=bass.IndirectOffsetOnAxis(ap=ids_sb[:, ti:ti + 1],
                                                    axis=0))
            we_t.append(g)
        z = act.tile([P, KD, S], BF, name="z", bufs=2)
        for f in range(KD):
            pst = ps.tile([P, S], BF, name="a", bufs=2)
            for ti in range(NT):
                nc.tensor.transpose(out=pst[:, ti * P:(ti + 1) * P],
                                    in_=we_t[ti][:, f * P:(f + 1) * P],
                                    identity=ident_bf[:])
            nc.vector.tensor_add(out=z[:, f, :], in0=pst[:],
                                 in1=posT_sb[:, f, :])

        def layer_norm(zt, g_sb, b_sb, name):
            """LN over the feature (partition) dim of zt [P, KD, S] -> bf16."""
            s1 = ps.tile([1, S], FP, name="b", bufs=2)
            s2 = ps.tile([1, S], FP, name="c", bufs=2)
            for c in range(KD):
                nc.tensor.matmul(out=s1[:], lhsT=ones_col[:], rhs=zt[:, c, :],
                                 start=(c == 0), stop=(c == KD - 1))
                sq = misc.tile([P, S], BF, name="sqs", bufs=2)
                nc.vector.tensor_mul(out=sq[:], in0=zt[:, c, :],
                                     in1=zt[:, c, :])
                nc.tensor.matmul(out=s2[:], lhsT=ones_col[:], rhs=sq[:],
                                 start=(c == 0), stop=(c == KD - 1))
            m2 = misc.tile([1, S], FP, name="m2")
            nc.scalar.activation(out=m2[:], in_=s1[:], func=AF.Square,
                                 scale=1.0 / math.sqrt(D))
            u = misc.tile([1, S], FP, name="u")
            nc.vector.tensor_tensor(out=u[:], in0=s2[:], in1=m2[:],
                                    op=mybir.AluOpType.subtract)
            r = misc.tile([1, S], FP, name="r")
            nc.scalar.activation(out=r[:], in_=u[:], func=AF.Sqrt,
                                 bias=eps_t[:], scale=1.0 / D)
            rstd = misc.tile([1, S], FP, name="rstd")
            nc.vector.reciprocal(out=rstd[:], in_=r[:])
            mu = misc.tile([1, S], FP, name="mu")
            nc.scalar.mul(out=mu[:], in_=s1[:], mul=1.0 / D)
            mu_b = ps.tile([P, S], FP, name="b", bufs=2)
            nc.tensor.matmul(out=mu_b[:], lhsT=ones_row[:], rhs=mu[:],
                             start=True, stop=True)
            rstd_b = ps.tile([P, S], FP, name="c", bufs=2)
            nc.tensor.matmul(out=rstd_b[:], lhsT=ones_row[:], rhs=rstd[:],
                             start=True, stop=True)
            xo = act.tile([P, KD, S], BF, name=name, bufs=2)
            for c in range(KD):
                tt = misc.tile([P, S], FP, name="lnt", bufs=2)
                nc.vector.tensor_tensor(out=tt[:], in0=zt[:, c, :],
                                        in1=mu_b[:],
                                        op=mybir.AluOpType.subtract)
                tt2 = misc.tile([P, S], FP, name="lnt2", bufs=2)
                nc.vector.tensor_mul(out=tt2[:], in0=tt[:], in1=rstd_b[:])
                nc.scalar.activation(
                    out=xo[:, c, :], in_=tt2[:], func=AF.Identity,
                    bias=b_sb[:, c:c + 1], scale=g_sb[:, c:c + 1])
            return xo

        xT = layer_norm(z, embg_sb, embb_sb, "xT")
        if debug_taps:
            dbg = act.tile([P, KD, S], FP, name="dbg", bufs=1)
            for c in range(KD):
                nc.vector.tensor_copy(out=dbg[:, c, :], in_=xT[:, c, :])
            nc.sync.dma_start(out=taps["x0"][:], in_=dbg[:])

        def col_bias(src, ncols, name):
            b_sb = misc.tile([P, ncols], FP, name=name)
            nc.sync.dma_start(out=b_sb[:],
                              in_=src.rearrange("(c p) 1 -> p c", p=P))
            return b_sb

        # ---- transformer layers ------------------------------------------
        for l in range(L):
            bq_sb = col_bias(t["bq"][l], KD, "bq_sb")
            bk_sb = col_bias(t["bk"][l], KD, "bk_sb")
            bo_sb = col_bias(t["bo"][l], KD, "bo_sb")
            b2_sb = col_bias(t["b2"][l], KD, "b2_sb")
            b1_sb = col_bias(t["b1"][l], KF, "b1_sb")
            l1g_sb = col_bias(t["ln1g"][l], KD, "l1g_sb")
            l1b_sb = col_bias(t["ln1b"][l], KD, "l1b_sb")
            l2g_sb = col_bias(t["ln2g"][l], KD, "l2g_sb")
            l2b_sb = col_bias(t["ln2b"][l], KD, "l2b_sb")
            bv_row = misc.tile([1, D], FP, name="bv_row")
            nc.sync.dma_start(out=bv_row[:], in_=t["bv"][l])

            # Q/K projections -> feature-major QT/KT [P, KD, S]
            wq_sb = wt.tile([P, KD, D], BF, name="wx", bufs=3)
            nc.sync.dma_start(out=wq_sb[:],
                              in_=t["wq"][l].rearrange("(c p) f -> p c f", p=P))
            wk_sb = wt.tile([P, KD, D], BF, name="wx", bufs=3)
            nc.sync.dma_start(out=wk_sb[:],
                              in_=t["wk"][l].rearrange("(c p) f -> p c f", p=P))
            qT = act.tile([P, KD, S], BF, name="qT")
            kT = act.tile([P, KD, S], BF, name="kT")
            for dst, w_sb, b_sb in ((qT, wq_sb, bq_sb), (kT, wk_sb, bk_sb)):
                for f in range(KD):
                    pst = ps.tile([P, S], FP, name="a", bufs=4)
                    for c in range(KD):
                        nc.tensor.matmul(out=pst[:],
                                         lhsT=w_sb[:, c, f * P:(f + 1) * P],
                                         rhs=xT[:, c, :],
                                         start=(c == 0), stop=(c == KD - 1))
                    nc.vector.tensor_scalar_add(out=dst[:, f, :], in0=pst[:],
                                                scalar1=b_sb[:, f:f + 1])

            # V projection -> token-major V_aug [P(tok), NT, H, DH+1]
            wv_sb = wt.tile([P, KD, D], BF, name="wx", bufs=3)
            nc.sync.dma_start(out=wv_sb[:],
                              in_=t["wv"][l].rearrange("(c p) f -> p c f", p=P))
            va = act.tile([P, NT, H, DH + 1], BF, name="va")
            nc.vector.memset(va[:, :, :, DH:DH + 1], 1.0)
            for ti in range(NT):
                for fb in range(2):
                    pst = ps.tile([P, 384], FP, name="b", bufs=2)
                    nc.tensor.matmul(
                        out=pst[:],
                        lhsT=ones_row[:],
                        rhs=bv_row[:, fb * 384:(fb + 1) * 384],
                        start=True, stop=False)
                    for c in range(KD):
                        nc.tensor.matmul(
                            out=pst[:],
                            lhsT=xT[:, c, ti * P:(ti + 1) * P],
                            rhs=wv_sb[:, c, fb * 384:(fb + 1) * 384],
                            start=False, stop=(c == KD - 1))
                    nc.vector.tensor_copy(
                        out=va[:, ti, fb * 6:(fb + 1) * 6, 0:DH],
                        in_=pst.rearrange("p (a b) -> p a b", a=6))

            # attention per head
            ctxT = act.tile([P, KD, S], BF, name="ctxT")
            for h in range(H):
                hp = (h % 2) * DH
                hf = h // 2
                expT = act.tile([P, NT, S], BF, name="expT", bufs=3)
                for kt in range(NT):
                    ps_s = ps.tile([P, S], FP, name="a", bufs=4)
                    nc.tensor.matmul(
                        out=ps_s[:],
                        lhsT=kT[hp:hp + DH, hf, kt * P:(kt + 1) * P],
                        rhs=qT[hp:hp + DH, hf, :],
                        start=True, stop=True)
                    nc.scalar.activation(
                        out=expT[:, kt, :], in_=ps_s[:], func=AF.Exp,
                        scale=1.0 / math.sqrt(DH),
                        bias=kb_sb[:, kt:kt + 1])
                ps_c = ps.tile([P, S], FP, name="c", bufs=2)
                for kt in range(NT):
                    nc.tensor.matmul(out=ps_c[:DH + 1, :],
                                     lhsT=va[:, kt, h, :],
                                     rhs=expT[:, kt, :],
                                     start=(kt == 0), stop=(kt == NT - 1))
                rec = misc.tile([1, S], FP, name="rec", bufs=2)
                nc.vector.reciprocal(out=rec[:], in_=ps_c[DH:DH + 1, :])
                rec_b = ps.tile([P, S], FP, name="b", bufs=2)
                nc.tensor.matmul(out=rec_b[:DH, :],
                                 lhsT=ones_row[:, :DH], rhs=rec[:],
                                 start=True, stop=True)
                craw = misc.tile([DH, S], FP, name="craw", bufs=2)
                nc.vector.tensor_copy(out=craw[:], in_=ps_c[:DH, :])
                nc.vector.tensor_tensor(out=ctxT[hp:hp + DH, hf, :],
                                        in0=craw[:], in1=rec_b[:DH, :],
                                        op=mybir.AluOpType.mult)

            # attention output projection + residual + LN1
            wo_sb = wt.tile([P, KD, D], BF, name="wx", bufs=3)
            nc.sync.dma_start(out=wo_sb[:],
                              in_=t["wo"][l].rearrange("(c p) f -> p c f", p=P))
            z1 = act.tile([P, KD, S], BF, name="z", bufs=2)
            for f in range(KD):
                pst = ps.tile([P, S], FP, name="a", bufs=4)
                for c in range(KD):
                    nc.tensor.matmul(out=pst[:],
                                     lhsT=wo_sb[:, c, f * P:(f + 1) * P],
                                     rhs=ctxT[:, c, :],
                                     start=(c == 0), stop=(c == KD - 1))
                nc.vector.tensor_scalar_add(out=pst[:], in0=pst[:],
                                            scalar1=bo_sb[:, f:f + 1])
                nc.vector.tensor_tensor(out=z1[:, f, :], in0=pst[:],
                                        in1=xT[:, f, :],
                                        op=mybir.AluOpType.add)
            x1 = layer_norm(z1, l1g_sb, l1b_sb, "xT")

            # FFN up-projection + gelu, W1 streamed in two halves
            hT = act.tile([P, KF, S], BF, name="hT")
            w1_sb = wt.tile([P, KD, DF], BF, name="w1h", bufs=1)
            nc.sync.dma_start(
                out=w1_sb[:],
                in_=t["w1"][l].rearrange("(c p) f -> p c f", p=P))
            for f in range(KF):
                pst = ps.tile([P, S], FP, name="a", bufs=4)
                for c in range(KD):
                    nc.tensor.matmul(out=pst[:],
                                     lhsT=w1_sb[:, c, f * P:(f + 1) * P],
                                     rhs=x1[:, c, :],
                                     start=(c == 0), stop=(c == KD - 1))
                nc.scalar.activation(out=hT[:, f, :], in_=pst[:],
                                     func=AF.Gelu,
                                     bias=b1_sb[:, f:f + 1], scale=1.0)

            # FFN down-projection: two passes of 3 concurrent psum banks,
            # streaming w2 k-tiles (w2 is read twice)
            z2 = act.tile([P, KD, S], BF, name="z", bufs=2)
            ps_f = [ps.tile([P, S], FP, name=n, bufs=(4 if n == "a" else 2))
                    for n in ("a", "a", "b", "b", "c", "c")]
            for c in range(KF):
                w2_sb = wt.tile([P, D], BF, name="w2_sb", bufs=8)
                nc.sync.dma_start(out=w2_sb[:],
                                  in_=t["w2"][l][c * P:(c + 1) * P, :])
                for f in range(KD):
                    nc.tensor.matmul(out=ps_f[f][:],
                                     lhsT=w2_sb[:, f * P:(f + 1) * P],
                                     rhs=hT[:, c, :],
                                     start=(c == 0), stop=(c == KF - 1))
            for f in range(KD):
                nc.vector.tensor_scalar_add(out=ps_f[f][:], in0=ps_f[f][:],
                                            scalar1=b2_sb[:, f:f + 1])
                nc.vector.tensor_tensor(out=z2[:, f, :], in0=ps_f[f][:],
                                        in1=x1[:, f, :],
                                        op=mybir.AluOpType.add)
            xT = layer_norm(z2, l2g_sb, l2b_sb, "xT")
            if debug_taps:
                dbg = act.tile([P, KD, S], FP, name="dbg", bufs=1)
                for c in range(KD):
                    nc.vector.tensor_copy(out=dbg[:, c, :], in_=xT[:, c, :])
                nc.sync.dma_start(out=taps[f"x{l + 1}"][:], in_=dbg[:])

        # ---- classifier + softmax + compaction ---------------------------
        clsw_sb = const.tile([P, KD, NL], BF)
        nc.sync.dma_start(out=clsw_sb[:],
                          in_=t["clsw"].rearrange("(c p) n -> p c n", p=P))
        clsb_sb = const.tile([NL, 1], FP)
        nc.sync.dma_start(out=clsb_sb[:], in_=t["clsb"][:])
        clsbr_sb = const.tile([1, NL], FP)
        nc.sync.dma_start(out=clsbr_sb[:], in_=t["clsb_row"][:])

        ps_l = ps.tile([NL, S], FP, name="a", bufs=2)
        for c in range(KD):
            nc.tensor.matmul(out=ps_l[:], lhsT=clsw_sb[:, c, :],
                             rhs=xT[:, c, :], start=(c == 0),
                             stop=(c == KD - 1))
        logitsT = misc.tile([NL, S], FP, name="logitsT")
        nc.vector.tensor_scalar_add(out=logitsT[:], in0=ps_l[:],
                                    scalar1=clsb_sb[:])
        if debug_taps:
            nc.sync.dma_start(out=taps["logitsT"][:], in_=logitsT[:])

        # transpose logits to token-major, softmax over the 9 classes
        probs = misc.tile([P, NT, NL], BF, name="probs")
        for ti in range(NT):
            ps_t = ps.tile([P, S], FP, name="b", bufs=2)
            nc.tensor.transpose(out=ps_t[:, :NL],
                                in_=logitsT[:, ti * P:(ti + 1) * P],
                                identity=ident_fp[:NL, :NL])
            ex = misc.tile([P, NL], FP, name="ex", bufs=2)
            den = misc.tile([P, 1], FP, name="den", bufs=2)
            nc.scalar.activation(out=ex[:], in_=ps_t[:, :NL], func=AF.Exp,
                                 accum_out=den[:])
            rden = misc.tile([P, 1], FP, name="rden", bufs=2)
            nc.vector.reciprocal(out=rden[:], in_=den[:])
            nc.vector.tensor_scalar_mul(out=probs[:, ti, :], in0=ex[:],
                                        scalar1=rden[:])
        # padding row: softmax(cls_b)
        exb = misc.tile([1, NL], FP, name="exb")
        denb = misc.tile([1, 1], FP, name="denb")
        nc.scalar.activation(out=exb[:], in_=clsbr_sb[:], func=AF.Exp,
                             accum_out=denb[:])
        rdenb = misc.tile([1, 1], FP, name="rdenb")
        nc.vector.reciprocal(out=rdenb[:], in_=denb[:])
        pad_probs = misc.tile([1, NL], BF, name="pad_probs")
        nc.vector.tensor_scalar_mul(out=pad_probs[:], in0=exb[:],
                                    scalar1=rdenb[:])

        # compaction via permutation matmul
        pmT_sb = wt.tile([P, 5, S], BF, name="bigscratch")
        nc.sync.dma_start(out=pmT_sb[:],
                          in_=t["pmT"].rearrange("(a p) s -> p a s", p=P))
        out_sb = misc.tile([P, NT, NL], FP, name="out_sb")
        for i in range(NT):
            ps_o = ps.tile([P, S], FP, name="c", bufs=2)
            for ti in range(NT):
                nc.tensor.matmul(out=ps_o[:, :NL],
                                 lhsT=pmT_sb[:, ti, i * P:(i + 1) * P],
                                 rhs=probs[:, ti, :],
                                 start=(ti == 0), stop=False)
            nc.tensor.matmul(out=ps_o[:, :NL],
                             lhsT=pmT_sb[0:1, 4, i * P:(i + 1) * P],
                             rhs=pad_probs[:],
                             start=False, stop=True)
            nc.vector.tensor_copy(out=out_sb[:, i, :], in_=ps_o[:, :NL])
        nc.sync.dma_start(out=t["outp"].rearrange("(i p) n -> p i n", p=P),
                          in_=out_sb[:])


_NC_CACHE = {}


def _get_nc(debug_taps=False):
    key = bool(debug_taps)
    if key not in _NC_CACHE:
        _NC_CACHE[key] = build_nc(debug_taps)
    return _NC_CACHE[key]


def make_in_maps(inputs):
    """Build the 8 per-core input maps from the full-problem inputs."""
    inp = {k: np.asarray(v) for k, v in inputs.items()}
    wemb_bf = inp["word_emb"].astype(BF_NP)
    shared = dict(
        wemb=wemb_bf,
        emb_g=inp["emb_g"].reshape(D, 1).astype(np.float32),
        emb_b=inp["emb_b"].reshape(D, 1).astype(np.float32),
        wq=inp["Wq"].astype(BF_NP), wk=inp["Wk"].astype(BF_NP),
        wv=inp["Wv"].astype(BF_NP), wo=inp["Wo"].astype(BF_NP),
        w1=inp["W1"].astype(BF_NP), w2=inp["W2"].astype(BF_NP),
        bq=inp["bq"].reshape(L, D, 1).astype(np.float32),
        bk=inp["bk"].reshape(L, D, 1).astype(np.float32),
        bv=inp["bv"].reshape(L, 1, D).astype(np.float32),
        bo=inp["bo"].reshape(L, D, 1).astype(np.float32),
        b1=inp["b1"].reshape(L, DF, 1).astype(np.float32),
        b2=inp["b2"].reshape(L, D, 1).astype(np.float32),
        ln1g=inp["ln1_g"].reshape(L, D, 1).astype(np.float32),
        ln1b=inp["ln1_b"].reshape(L, D, 1).astype(np.float32),
        ln2g=inp["ln2_g"].reshape(L, D, 1).astype(np.float32),
        ln2b=inp["ln2_b"].reshape(L, D, 1).astype(np.float32),
        clsw=inp["cls_W"].astype(BF_NP),
        clsb=inp["cls_b"].reshape(NL, 1).astype(np.float32),
        clsb_row=inp["cls_b"].reshape(1, NL).astype(np.float32),
    )
    in_maps = []
    for c in range(8):
        b = c // 2
        pt = (inp["pos_emb"] + inp["type_emb"][inp["input_type_ids"][b]])
        posT_b = np.ascontiguousarray(pt.T).astype(BF_NP)
        kbias_b = ((inp["input_mask"][b].astype(np.float32) - 1.0)
                   * 60.0).reshape(S, 1)
        vm = inp["valid_mask"][b]
        order = np.argsort(1 - vm, kind="stable")
        n_valid = int(vm.sum())
        pm = np.zeros((5 * P, S), dtype=BF_NP)
        for i in range(S):
            if i < n_valid:
                pm[order[i], i] = 1
            else:
                pm[S, i] = 1
        in_maps.append(dict(
            shared,
            ids=inp["input_word_ids"][b].reshape(S, 1).astype(np.int32),
            posT=posT_b,
            kbias=kbias_b.astype(np.float32),
            pmT=pm,
        ))
    return in_maps


def kernel(**inputs) -> np.ndarray:
    nc = _get_nc()
    in_maps = make_in_maps(inputs)
    res = run_bass_kernel_spmd(nc, in_maps, core_ids=list(range(8)))
    out = np.stack([res.results[2 * b]["out"] for b in range(B)], axis=0)
    return out.astype(np.float32)
